# revision 12
# baseline (speedup 1.0000x reference)
"""Trainium2 Bass kernel for nn_ConformerMHSAV3 (LayerNorm + packed-QKV MHSA,
online/causal + offline/full-context variants, stacked output).

Sharding: 8 cores = 4 batches x 2 head-groups (8 heads each).  Each core
computes LN + its head-group's QKV + attention (both variants) + a partial
output projection; the host sums the two head-group partials per batch and
adds the output bias.

Everything runs in fp32 with fp32r matmuls (full PE rate at N>=256).
Softmax uses a constant shift instead of a row max (scores are O(1) after
LayerNorm + 1/sqrt scaling), with key-padding folded into the exp bias and
the attention mask applied as block-sparse 0/1 multiplies only where the
mask block is mixed (computed from the actual mask at build time, so any
mask pattern is handled; causal gets the fast path).
"""

import numpy as np

import concourse.bass as bass
import concourse.mybir as mybir
import concourse.tile as tile
from concourse import mybir as _mybir
from concourse.bass_utils import run_bass_kernel_spmd
from concourse.vector_clock import ScopedClock, VectorClock

# ---------------------------------------------------------------------------
# Patches for this walrus build's 1-sync-wait-per-instruction cap.
# ---------------------------------------------------------------------------

_MAX_WAITS = 1


def _drain_and_barrier(self, tick_clock, wait_clock):
    gc = ScopedClock({None: tick_clock.global_clock})[None]
    n = len(gc)
    for p in [i for i in range(n) if gc[i] > 0]:
        nop = self.nc.sync.nop(nofuse=True, hint="tail_drain_split")
        partial = VectorClock([gc[j] if j == p else 0 for j in range(n)])
        wait_clock.add_sem_waits(nop.ins, ScopedClock({None: partial}))
    self.nc.sync.drain()
    self.nc.all_engine_barrier()
    assert self.sems is not None
    popped = self.nc._tile_sem_poison_stack.pop()
    assert popped is self._sem_poison
    self.nc.clear_and_free_semaphores(list(self.sems.allocated().values()))
    self.nc.all_engine_barrier()


def _install_patches():
    tile.TileContext._drain_and_barrier = _drain_and_barrier


def _split_multi_waits(nc):
    """Move all-but-one sem wait of each instruction onto same-engine NOPs
    inserted immediately before it (preserves per-engine program order)."""
    for f in nc.m.functions:
        for bb in f.blocks:
            insts = bb.instructions
            i = 0
            while i < len(insts):
                inst = insts[i]
                si = inst.sync_info
                if si is not None and si.on_wait and len(si.on_wait) > _MAX_WAITS:
                    extra = []
                    while len(si.on_wait) > _MAX_WAITS:
                        extra.append(si.on_wait.pop())
                    for w in extra:
                        nop = nc.engines[inst.engine].nop(nofuse=True).ins
                        for blk in f.blocks:
                            if blk.instructions and blk.instructions[-1] is nop:
                                blk.instructions.pop()
                                break
                        if nop.sync_info is None:
                            nop.sync_info = _mybir.SyncInfo(on_wait=[w], on_update=[])
                        else:
                            nop.sync_info.on_wait.append(w)
                        insts.insert(i, nop)
                        i += 1
                i += 1


# ---------------------------------------------------------------------------
# Problem constants (hardcoded per the self-contained-kernel contract).
# ---------------------------------------------------------------------------

B, T, D, H = 4, 1024, 1024, 16
HD = D // H          # 64
HL = H // 2          # 8 local heads per core
P = 128
NT = T // P          # 8 tiles of 128
EPS = 1e-5
C_SHIFT = 12.0       # constant softmax shift (exact-softmax invariant)
NEG = -1e30
F32 = mybir.dt.float32
F32R = mybir.dt.float32r
QB = 256             # online q-block granularity
NQB = T // QB        # 4

_prog_cache = {}


def _classify_blocks(attnT):
    """Per (k-chunk, q-half) classification of the online attention mask at
    PSUM-bank granularity (512 columns), so each bank hosts exactly one
    accumulation group (start=True clears has_written for the whole bank).
    Returns (cls[c][h] in {0:none,1:full,2+idx:masked}, additive blocks)."""
    cls = [[0] * 2 for _ in range(NT)]
    mixed = []
    for c in range(NT):
        for h in range(2):
            blk = attnT[c * P : (c + 1) * P, h * 512 : (h + 1) * 512]
            if blk.all():
                cls[c][h] = 1
            elif not blk.any():
                cls[c][h] = 0
            else:
                cls[c][h] = 2 + len(mixed)
                mixed.append(np.where(blk, 0.0, NEG).astype(np.float32))
    return cls, mixed


def _build_program(used_chunks, cls, n_mixed):
    """Build the SPMD Bass program.  Structure depends only on the masks'
    block classification, which is identical across cores."""
    nc = bass.Bass("TRN2", target_bir_lowering=False, debug=False)

    x_d = nc.declare_dram_parameter("x", [T, D], F32, isOutput=False)
    xT_d = nc.declare_dram_parameter("xT", [D, T], F32, isOutput=False)
    wqkT_d = nc.declare_dram_parameter("wqkT", [D, 2 * HL * HD], F32, isOutput=False)
    wvT_d = nc.declare_dram_parameter("wvT", [D, HL * HD], F32, isOutput=False)
    woT_d = nc.declare_dram_parameter("woT", [HL * HD, D], F32, isOutput=False)
    bqk_d = nc.declare_dram_parameter("bqk", [2 * HL * HD], F32, isOutput=False)
    bv_d = nc.declare_dram_parameter("bv", [HL * HD], F32, isOutput=False)
    seqb_d = nc.declare_dram_parameter("seqb", [P, NT], F32, isOutput=False)
    nmx = max(n_mixed, 1)
    mix_d = nc.declare_dram_parameter("mix", [nmx, P, 512], F32, isOutput=False)
    oon_d = nc.declare_dram_parameter("out_on", [T, D], F32, isOutput=True)
    ooff_d = nc.declare_dram_parameter("out_off", [T, D], F32, isOutput=True)

    ACT = mybir.ActivationFunctionType
    OP = mybir.AluOpType

    first_off, last_off = used_chunks[0], used_chunks[-1]
    first_on = [None, None]
    last_on = [None, None]
    for qh in range(2):
        writers = [c for c in used_chunks if cls[c][qh] != 0]
        if writers:
            first_on[qh], last_on[qh] = writers[0], writers[-1]

    with tile.TileContext(nc) as tc:
        with (
            tc.tile_pool(name="io", bufs=2) as p_io,
            tc.tile_pool(name="big", bufs=2) as p_big,
            tc.tile_pool(name="w1", bufs=1) as p_w1,
            tc.tile_pool(name="w2", bufs=1) as p_w2,
            tc.tile_pool(name="qk", bufs=1) as p_qk,
            tc.tile_pool(name="vv", bufs=1) as p_v,
            tc.tile_pool(name="pp", bufs=3) as p_p,
            tc.tile_pool(name="blk", bufs=2) as p_blk,
            tc.tile_pool(name="bc", bufs=2) as p_bc,
            tc.tile_pool(name="rc", bufs=2) as p_rc,
            tc.tile_pool(name="sm", bufs=1) as p_sm,
            tc.tile_pool(name="st", bufs=3) as p_st,
            tc.tile_pool(name="dram", bufs=2, space="DRAM") as p_dram,
            tc.tile_pool(name="ps", bufs=2, space="PSUM") as p_ps,
            tc.tile_pool(name="po", bufs=2, space="PSUM") as p_po,
        ):
            # ---------------- Phase A: LN statistics --------------------
            mu_all = p_sm.tile([P, NT], F32, tag="mu")
            rstd_all = p_sm.tile([P, NT], F32, tag="rstd")
            mun_all = p_sm.tile([P, NT], F32, tag="mun")
            eps_t = p_sm.tile([P, 1], F32, tag="eps")
            nc.vector.memset(eps_t, EPS)

            for t in range(NT):
                xt = p_io.tile([P, D], F32, tag="io")
                nc.sync.dma_start(out=xt, in_=x_d[t * P : (t + 1) * P, :])
                stats = p_st.tile([P, 2, 6], F32, tag="bst")
                xv = xt.rearrange("p (s q) -> p s q", s=2)
                for s in range(2):
                    nc.vector.bn_stats(out=stats[:, s, :], in_=xv[:, s, :])
                mv = p_st.tile([P, 2], F32, tag="mv")
                nc.vector.bn_aggr(out=mv, in_=stats)
                nc.gpsimd.tensor_copy(out=mu_all[:, t : t + 1], in_=mv[:, 0:1])
                # rstd = 1/sqrt(var + eps)  (same recipe as prod groupnorm)
                nc.scalar.activation(
                    out=rstd_all[:, t : t + 1], in_=mv[:, 1:2],
                    func=ACT.Sqrt, bias=eps_t, scale=1.0,
                )
            nc.vector.reciprocal(out=rstd_all, in_=rstd_all)
            nc.vector.tensor_tensor(out=mun_all, in0=mu_all, in1=rstd_all, op=OP.mult)
            nc.scalar.mul(out=mun_all, in_=mun_all, mul=-1.0)

            # stripe [P, NT] -> DRAM rows (t = 128*tile + p ordering)
            scr = p_dram.tile([2, T], F32, tag="scr")
            nc.sync.dma_start(
                out=scr[0].rearrange("(n p) -> p n", p=P), in_=rstd_all
            )
            nc.sync.dma_start(
                out=scr[1].rearrange("(n p) -> p n", p=P), in_=mun_all
            )
            # broadcast rows across all 128 partitions
            rstd_bc = p_bc.tile([P, T], F32, tag="bc")
            mun_bc = p_bc.tile([P, T], F32, tag="bc")
            row0 = bass.AP(tensor=scr.tensor, offset=scr.offset, ap=[[0, P], [1, T]])
            row1 = bass.AP(tensor=scr.tensor, offset=scr.offset + T, ap=[[0, P], [1, T]])
            nc.sync.dma_start(out=rstd_bc, in_=row0)
            nc.sync.dma_start(out=mun_bc, in_=row1)

            # ---------------- Phase B: normalized transpose -------------
            # xnT[d, t] = xT[d, t] * rstd[t] + (-mu[t]*rstd[t])
            xnT_a = p_big.tile([P, 4, T], F32R, tag="big")
            xnT_b = p_big.tile([P, 4, T], F32R, tag="big")

            def xnT_sl(ko):
                return (xnT_a if ko < 4 else xnT_b)[:, ko % 4, :]

            for ko in range(NT):
                xtt = p_io.tile([P, T], F32, tag="io")
                nc.sync.dma_start(out=xtt, in_=xT_d[ko * P : (ko + 1) * P, :])
                tmp = p_st.tile([P, T], F32, tag="xtmp")
                nc.gpsimd.tensor_tensor(out=tmp, in0=xtt, in1=rstd_bc, op=OP.mult)
                nc.vector.tensor_tensor(out=xnT_sl(ko), in0=tmp, in1=mun_bc, op=OP.add)

            # ---------------- Phase C: qkT = Wqk' @ xn^T ----------------
            wqk_sb = p_w1.tile([P, NT, 2 * HL * HD], F32R, tag="w1")
            nc.sync.dma_start(
                out=wqk_sb, in_=wqkT_d[:].rearrange("(ko p) m -> p ko m", p=P).bitcast(F32R)
            )
            bqk_sb = p_sm.tile([P, NT], F32, tag="bqk")
            nc.sync.dma_start(out=bqk_sb, in_=bqk_d[:].rearrange("(mt p) -> p mt", p=P))
            qkT_sb = p_qk.tile([P, NT, T], F32R, tag="qk")

            for mt in range(NT):
                psq = p_ps.tile([P, T], F32, tag="ps")
                for qh in range(2):
                    for ko in range(NT):
                        nc.tensor.matmul(
                            psq[:, qh * 512 : (qh + 1) * 512],
                            lhsT=wqk_sb[:, ko, mt * P : (mt + 1) * P],
                            rhs=xnT_sl(ko)[:, qh * 512 : (qh + 1) * 512],
                            start=(ko == 0),
                            stop=(ko == NT - 1),
                        )
                nc.scalar.activation(
                    out=qkT_sb[:, mt, :], in_=psq,
                    func=ACT.Identity, bias=bqk_sb[:, mt : mt + 1], scale=1.0,
                )

            # ---------------- Phase D: v (head-interleaved, ones col) ---
            wv_sb = p_w2.tile([P, NT, HL * HD], F32R, tag="w2")
            nc.sync.dma_start(
                out=wv_sb, in_=wvT_d[:].rearrange("(ko p) m -> p ko m", p=P).bitcast(F32R)
            )
            bv_bc = p_st.tile([P, HL * HD], F32, tag="bvbc")
            nc.sync.dma_start(
                out=bv_bc,
                in_=bass.AP(tensor=bv_d, offset=0, ap=[[0, P], [1, HL * HD]]),
            )
            # v_sb[p, c, 65h + j]: j<64 -> v head h dim j; j=64 -> 1.0
            v_sb = p_v.tile([P, NT, HL * 65], F32R, tag="vv")
            nc.vector.memset(
                v_sb.rearrange("p c (h j) -> p c h j", j=65)[:, :, :, 64].bitcast(F32),
                1.0,
            )
            for t in range(NT):
                psv = p_ps.tile([P, T], F32, tag="ps")
                for ko in range(NT):
                    nc.tensor.matmul(
                        psv[:, : HL * HD],
                        lhsT=xnT_sl(ko)[:, t * P : (t + 1) * P],
                        rhs=wv_sb[:, ko, :],
                        start=(ko == 0),
                        stop=(ko == NT - 1),
                    )
                nc.vector.tensor_tensor(
                    out=v_sb.rearrange("p c (h j) -> p c h j", j=65)[:, t, :, 0:64],
                    in0=psv[:, : HL * HD].rearrange("p (h j) -> p h j", j=HD),
                    in1=bv_bc.rearrange("p (h j) -> p h j", j=HD),
                    op=OP.add,
                )

            # ---------------- Phase E: attention per head ---------------
            seqb_sb = p_sm.tile([P, NT], F32, tag="seqb")
            nc.sync.dma_start(out=seqb_sb, in_=seqb_d[:])
            mix_sb = p_w2.tile([P, nmx, 512], F32, tag="mix")
            nc.sync.dma_start(
                out=mix_sb, in_=mix_d[:].rearrange("n p q -> p n q")
            )

            oT_on = p_big.tile([P, 4, T], F32R, tag="big")
            oT_off = p_big.tile([P, 4, T], F32R, tag="big")

            for h in range(HL):
                par = h % 2
                base = 64 * par
                qT_h = qkT_sb[base : base + 64, h // 2, :]
                kT_h = qkT_sb[base : base + 64, 4 + h // 2, :]
                vlo = 65 * h
                pon_t = p_po.tile([P, T], F32, tag="po")
                poff_t = p_po.tile([P, T], F32, tag="po")
                pon = pon_t[0:65]
                poff = poff_t[0:65]

                for ci, c in enumerate(used_chunks):
                    pss = p_ps.tile([P, T], F32, tag="ps")
                    for qh in range(2):
                        nc.tensor.matmul(
                            pss[:, qh * 512 : (qh + 1) * 512],
                            lhsT=kT_h[:, c * P : (c + 1) * P],
                            rhs=qT_h[:, qh * 512 : (qh + 1) * 512],
                            start=True,
                            stop=True,
                        )
                    pofc = p_p.tile([P, T], F32R, tag="pp")
                    nc.scalar.activation(
                        out=pofc, in_=pss, func=ACT.Exp,
                        bias=seqb_sb[:, c : c + 1], scale=1.0,
                    )
                    lhsT = v_sb[:, c, vlo : vlo + 65]
                    for qh in range(2):
                        nc.tensor.matmul(
                            poff[:, qh * 512 : (qh + 1) * 512],
                            lhsT=lhsT,
                            rhs=pofc[:, qh * 512 : (qh + 1) * 512],
                            start=(c == first_off),
                            stop=(c == last_off),
                        )
                    for qh in range(2):
                        k = cls[c][qh]
                        if k == 0 or first_on[qh] is None:
                            continue
                        qsl = slice(qh * 512, (qh + 1) * 512)
                        if k == 1:
                            rhs = pofc[:, qsl]
                        else:
                            nc.vector.tensor_tensor(
                                out=pss[:, qsl],
                                in0=pss[:, qsl],
                                in1=mix_sb[:, k - 2, :],
                                op=OP.add,
                            )
                            pblk = p_blk.tile([P, 512], F32R, tag="blk")
                            nc.scalar.activation(
                                out=pblk, in_=pss[:, qsl], func=ACT.Exp,
                                bias=seqb_sb[:, c : c + 1], scale=1.0,
                            )
                            rhs = pblk
                        nc.tensor.matmul(
                            pon[:, qsl],
                            lhsT=lhsT,
                            rhs=rhs,
                            start=(c == first_on[qh]),
                            stop=(c == last_on[qh]),
                        )

                # divide by the ones-column sums; DVE handles the partition
                # shift to this head's lanes (dlo) directly.
                dlo = base
                for pt, dst in ((pon_t, oT_on), (poff_t, oT_off)):
                    dbt = p_bc.tile([P, T], F32, tag="bc")
                    nc.scalar.activation(
                        out=dbt[64:65, :], in_=pt[64:65], func=ACT.Copy
                    )
                    drow_dram = p_dram.tile([T], F32, tag="drow")
                    nc.sync.dma_start(out=drow_dram[None, :], in_=dbt[64:65, :])
                    nc.sync.dma_start(
                        out=dbt[0:64, :],
                        in_=bass.AP(
                            tensor=drow_dram.tensor,
                            offset=drow_dram.offset,
                            ap=[[0, 64], [1, T]],
                        ),
                    )
                    rct = p_rc.tile([P, T], F32, tag="rc")
                    nc.vector.reciprocal(out=rct[0:64, :], in_=dbt[0:64, :])
                    nc.vector.tensor_tensor(
                        out=dst[dlo : dlo + 64, h // 2, :],
                        in0=pt[0:64],
                        in1=rct[0:64, :],
                        op=OP.mult,
                    )
                # zero any online q-halves no chunk wrote (fully masked)
                for qh in range(2):
                    if first_on[qh] is None:
                        nc.vector.memset(
                            oT_on[dlo : dlo + 64, h // 2, qh * 512 : (qh + 1) * 512]
                            .bitcast(F32),
                            0.0,
                        )

            # ---------------- Phase F: output projection ----------------
            wo_sb = p_w1.tile([P, 4, D], F32R, tag="w1")
            nc.sync.dma_start(
                out=wo_sb, in_=woT_d[:].rearrange("(j p) m -> p j m", p=P).bitcast(F32R)
            )
            for src, dst_d in ((oT_on, oon_d), (oT_off, ooff_d)):
                for t in range(NT):
                    pso = p_ps.tile([P, T], F32, tag="ps")
                    for dh in range(2):
                        for j in range(4):
                            nc.tensor.matmul(
                                pso[:, dh * 512 : (dh + 1) * 512],
                                lhsT=src[:, j, t * P : (t + 1) * P],
                                rhs=wo_sb[:, j, dh * 512 : (dh + 1) * 512],
                                start=(j == 0),
                                stop=(j == 3),
                            )
                    ot = p_io.tile([P, D], F32, tag="io")
                    nc.scalar.activation(out=ot, in_=pso, func=ACT.Copy)
                    nc.sync.dma_start(out=dst_d[t * P : (t + 1) * P, :], in_=ot)

    _split_multi_waits(nc)
    return nc


def _get_program(key, used_chunks, cls, n_mixed):
    if key not in _prog_cache:
        _install_patches()
        _prog_cache[key] = _build_program(used_chunks, cls, n_mixed)
    return _prog_cache[key]


def kernel(
    input_tensor,
    ln_gamma,
    ln_beta,
    in_proj_w,
    in_proj_b,
    out_w,
    out_b,
    sequence_mask,
    attn_mask,
):
    x = np.asarray(input_tensor, np.float32)
    gamma = np.asarray(ln_gamma, np.float32)
    beta = np.asarray(ln_beta, np.float32)
    W = np.asarray(in_proj_w, np.float32)
    bias = np.asarray(in_proj_b, np.float32)
    Wo = np.asarray(out_w, np.float32)
    bo = np.asarray(out_b, np.float32)
    seqm = np.asarray(sequence_mask, bool)
    attn = np.asarray(attn_mask, bool)

    # ---- mask-derived program structure (identical across cores) ----
    used_chunks = [
        c for c in range(NT) if seqm[:, c * P : (c + 1) * P].any()
    ] or [0]
    attnT = attn.T
    cls, mixed = _classify_blocks(attnT)
    key = (tuple(used_chunks), tuple(tuple(r) for r in cls))
    nc = _get_program(key, used_chunks, cls, len(mixed))

    if mixed:
        mix_arr = np.stack(mixed, axis=0)
    else:
        mix_arr = np.zeros((1, P, 512), np.float32)

    # ---- host-side weight folding (gamma/beta/scale into W, b) ----
    scale_q = 1.0 / np.sqrt(HD)
    Wg = W * gamma[None, :]          # fold gamma
    bfold = bias + W @ beta          # fold beta
    in_maps = []
    for c in range(8):
        b = c // 2
        g = c % 2
        qs, ks, vs = 512 * g, D + 512 * g, 2 * D + 512 * g
        wq = Wg[qs : qs + 512] * scale_q
        wk = Wg[ks : ks + 512]
        wv = Wg[vs : vs + 512]
        bq = bfold[qs : qs + 512] * scale_q
        bk = bfold[ks : ks + 512]
        bv = bfold[vs : vs + 512]
        wqkT = np.ascontiguousarray(np.concatenate([wq, wk], axis=0).T)
        seqb = np.where(seqm[b], 0.0, NEG).astype(np.float32) - C_SHIFT
        in_maps.append(
            {
                "x": np.ascontiguousarray(x[b]),
                "xT": np.ascontiguousarray(x[b].T),
                "wqkT": wqkT,
                "wvT": np.ascontiguousarray(wv.T),
                "woT": np.ascontiguousarray(Wo[:, 512 * g : 512 * g + 512].T),
                "bqk": np.ascontiguousarray(np.concatenate([bq, bk])),
                "bv": np.ascontiguousarray(bv),
                "seqb": np.ascontiguousarray(seqb.reshape(NT, P).T),
                "mix": mix_arr,
            }
        )

    global _last_in_maps
    _last_in_maps = in_maps
    res = run_bass_kernel_spmd(nc, in_maps, list(range(8)))

    out = np.empty((2, B, T, D), np.float32)
    for b in range(B):
        r0, r1 = res.results[2 * b], res.results[2 * b + 1]
        out[0, b] = r0["out_on"] + r1["out_on"] + bo[None, :]
        out[1, b] = r0["out_off"] + r1["out_off"] + bo[None, :]
    return out


# revision 13
# speedup vs baseline: 1.0355x; 1.0355x over previous
"""Trainium2 Bass kernel for nn_ConformerMHSAV3 (LayerNorm + packed-QKV MHSA,
online/causal + offline/full-context variants, stacked output).

Sharding: 8 cores = 4 batches x 2 head-groups (8 heads each).  Each core
computes LN + its head-group's QKV + attention (both variants) + a partial
output projection; the host sums the two head-group partials per batch and
adds the output bias.

Everything runs in fp32 with fp32r matmuls (full PE rate at N>=256).
Softmax uses a constant shift instead of a row max (scores are O(1) after
LayerNorm + 1/sqrt scaling), with key-padding folded into the exp bias and
the attention mask applied as block-sparse 0/1 multiplies only where the
mask block is mixed (computed from the actual mask at build time, so any
mask pattern is handled; causal gets the fast path).
"""

import numpy as np

import concourse.bass as bass
import concourse.mybir as mybir
import concourse.tile as tile
from concourse import mybir as _mybir
from concourse.bass_utils import run_bass_kernel_spmd
from concourse.vector_clock import ScopedClock, VectorClock

# ---------------------------------------------------------------------------
# Patches for this walrus build's 1-sync-wait-per-instruction cap.
# ---------------------------------------------------------------------------

_MAX_WAITS = 1


def _drain_and_barrier(self, tick_clock, wait_clock):
    gc = ScopedClock({None: tick_clock.global_clock})[None]
    n = len(gc)
    for p in [i for i in range(n) if gc[i] > 0]:
        nop = self.nc.sync.nop(nofuse=True, hint="tail_drain_split")
        partial = VectorClock([gc[j] if j == p else 0 for j in range(n)])
        wait_clock.add_sem_waits(nop.ins, ScopedClock({None: partial}))
    self.nc.sync.drain()
    self.nc.all_engine_barrier()
    assert self.sems is not None
    popped = self.nc._tile_sem_poison_stack.pop()
    assert popped is self._sem_poison
    self.nc.clear_and_free_semaphores(list(self.sems.allocated().values()))
    self.nc.all_engine_barrier()


def _install_patches():
    tile.TileContext._drain_and_barrier = _drain_and_barrier


def _split_multi_waits(nc):
    """Move all-but-one sem wait of each instruction onto same-engine NOPs
    inserted immediately before it (preserves per-engine program order)."""
    for f in nc.m.functions:
        for bb in f.blocks:
            insts = bb.instructions
            i = 0
            while i < len(insts):
                inst = insts[i]
                si = inst.sync_info
                if si is not None and si.on_wait and len(si.on_wait) > _MAX_WAITS:
                    extra = []
                    while len(si.on_wait) > _MAX_WAITS:
                        extra.append(si.on_wait.pop())
                    for w in extra:
                        nop = nc.engines[inst.engine].nop(nofuse=True).ins
                        for blk in f.blocks:
                            if blk.instructions and blk.instructions[-1] is nop:
                                blk.instructions.pop()
                                break
                        if nop.sync_info is None:
                            nop.sync_info = _mybir.SyncInfo(on_wait=[w], on_update=[])
                        else:
                            nop.sync_info.on_wait.append(w)
                        insts.insert(i, nop)
                        i += 1
                i += 1


# ---------------------------------------------------------------------------
# Problem constants (hardcoded per the self-contained-kernel contract).
# ---------------------------------------------------------------------------

B, T, D, H = 4, 1024, 1024, 16
HD = D // H          # 64
HL = H // 2          # 8 local heads per core
P = 128
NT = T // P          # 8 tiles of 128
EPS = 1e-5
C_SHIFT = 12.0       # constant softmax shift (exact-softmax invariant)
NEG = -1e30
F32 = mybir.dt.float32
F32R = mybir.dt.float32r
QB = 256             # online q-block granularity
NQB = T // QB        # 4

_prog_cache = {}


def _classify_blocks(attnT):
    """Per (k-chunk, q-half) classification of the online attention mask at
    PSUM-bank granularity (512 columns), so each bank hosts exactly one
    accumulation group (start=True clears has_written for the whole bank).
    Returns (cls[c][h] in {0:none,1:full,2+idx:masked}, additive blocks)."""
    cls = [[0] * 2 for _ in range(NT)]
    mixed = []
    for c in range(NT):
        for h in range(2):
            blk = attnT[c * P : (c + 1) * P, h * 512 : (h + 1) * 512]
            if blk.all():
                cls[c][h] = 1
            elif not blk.any():
                cls[c][h] = 0
            else:
                cls[c][h] = 2 + len(mixed)
                mixed.append(np.where(blk, 0.0, NEG).astype(np.float32))
    return cls, mixed


def _build_program(used_chunks, cls, n_mixed):
    """Build the SPMD Bass program.  Structure depends only on the masks'
    block classification, which is identical across cores."""
    nc = bass.Bass("TRN2", target_bir_lowering=False, debug=False)

    x_d = nc.declare_dram_parameter("x", [T, D], F32, isOutput=False)
    xT_d = nc.declare_dram_parameter("xT", [D, T], F32, isOutput=False)
    wqkT_d = nc.declare_dram_parameter("wqkT", [D, 2 * HL * HD], F32, isOutput=False)
    wvT_d = nc.declare_dram_parameter("wvT", [D, HL * HD], F32, isOutput=False)
    woT_d = nc.declare_dram_parameter("woT", [HL * HD, D], F32, isOutput=False)
    bqk_d = nc.declare_dram_parameter("bqk", [2 * HL * HD], F32, isOutput=False)
    bv_d = nc.declare_dram_parameter("bv", [HL * HD], F32, isOutput=False)
    seqb_d = nc.declare_dram_parameter("seqb", [P, NT], F32, isOutput=False)
    nmx = max(n_mixed, 1)
    mix_d = nc.declare_dram_parameter("mix", [nmx, P, 512], F32, isOutput=False)
    oon_d = nc.declare_dram_parameter("out_on", [T, D], F32, isOutput=True)
    ooff_d = nc.declare_dram_parameter("out_off", [T, D], F32, isOutput=True)

    ACT = mybir.ActivationFunctionType
    OP = mybir.AluOpType

    first_off, last_off = used_chunks[0], used_chunks[-1]
    first_on = [None, None]
    last_on = [None, None]
    for qh in range(2):
        writers = [c for c in used_chunks if cls[c][qh] != 0]
        if writers:
            first_on[qh], last_on[qh] = writers[0], writers[-1]

    with tile.TileContext(nc) as tc:
        with (
            tc.tile_pool(name="io", bufs=2) as p_io,
            tc.tile_pool(name="big", bufs=2) as p_big,
            tc.tile_pool(name="w1", bufs=1) as p_w1,
            tc.tile_pool(name="w2", bufs=1) as p_w2,
            tc.tile_pool(name="qk", bufs=1) as p_qk,
            tc.tile_pool(name="vv", bufs=1) as p_v,
            tc.tile_pool(name="pp", bufs=3) as p_p,
            tc.tile_pool(name="blk", bufs=2) as p_blk,
            tc.tile_pool(name="bc", bufs=2) as p_bc,
            tc.tile_pool(name="rc", bufs=2) as p_rc,
            tc.tile_pool(name="sm", bufs=1) as p_sm,
            tc.tile_pool(name="st", bufs=3) as p_st,
            tc.tile_pool(name="dram", bufs=2, space="DRAM") as p_dram,
            tc.tile_pool(name="ps", bufs=2, space="PSUM") as p_ps,
            tc.tile_pool(name="po", bufs=2, space="PSUM") as p_po,
        ):
            # ---------------- Phase A: LN statistics --------------------
            mu_all = p_sm.tile([P, NT], F32, tag="mu")
            rstd_all = p_sm.tile([P, NT], F32, tag="rstd")
            mun_all = p_sm.tile([P, NT], F32, tag="mun")
            eps_t = p_sm.tile([P, 1], F32, tag="eps")
            nc.vector.memset(eps_t, EPS)

            for t in range(NT):
                xt = p_io.tile([P, D], F32, tag="io")
                nc.sync.dma_start(out=xt, in_=x_d[t * P : (t + 1) * P, :])
                stats = p_st.tile([P, 2, 6], F32, tag="bst")
                xv = xt.rearrange("p (s q) -> p s q", s=2)
                for s in range(2):
                    nc.vector.bn_stats(out=stats[:, s, :], in_=xv[:, s, :])
                mv = p_st.tile([P, 2], F32, tag="mv")
                nc.vector.bn_aggr(out=mv, in_=stats)
                nc.gpsimd.tensor_copy(out=mu_all[:, t : t + 1], in_=mv[:, 0:1])
                # rstd = 1/sqrt(var + eps)  (same recipe as prod groupnorm)
                nc.scalar.activation(
                    out=rstd_all[:, t : t + 1], in_=mv[:, 1:2],
                    func=ACT.Sqrt, bias=eps_t, scale=1.0,
                )
            nc.vector.reciprocal(out=rstd_all, in_=rstd_all)
            nc.vector.tensor_tensor(out=mun_all, in0=mu_all, in1=rstd_all, op=OP.mult)
            nc.scalar.mul(out=mun_all, in_=mun_all, mul=-1.0)

            # stripe [P, NT] -> DRAM rows (t = 128*tile + p ordering)
            scr = p_dram.tile([2, T], F32, tag="scr")
            nc.sync.dma_start(
                out=scr[0].rearrange("(n p) -> p n", p=P), in_=rstd_all
            )
            nc.sync.dma_start(
                out=scr[1].rearrange("(n p) -> p n", p=P), in_=mun_all
            )
            # broadcast rows across all 128 partitions
            rstd_bc = p_bc.tile([P, T], F32, tag="bc")
            mun_bc = p_bc.tile([P, T], F32, tag="bc")
            row0 = bass.AP(tensor=scr.tensor, offset=scr.offset, ap=[[0, P], [1, T]])
            row1 = bass.AP(tensor=scr.tensor, offset=scr.offset + T, ap=[[0, P], [1, T]])
            nc.sync.dma_start(out=rstd_bc, in_=row0)
            nc.sync.dma_start(out=mun_bc, in_=row1)

            # ---------------- Phase B: normalized transpose -------------
            # xnT[d, t] = xT[d, t] * rstd[t] + (-mu[t]*rstd[t])
            xnT_a = p_big.tile([P, 4, T], F32R, tag="big")
            xnT_b = p_big.tile([P, 4, T], F32R, tag="big")

            def xnT_sl(ko):
                return (xnT_a if ko < 4 else xnT_b)[:, ko % 4, :]

            for ko in range(NT):
                xtt = p_io.tile([P, T], F32, tag="io")
                nc.sync.dma_start(out=xtt, in_=xT_d[ko * P : (ko + 1) * P, :])
                tmp = p_st.tile([P, T], F32, tag="xtmp")
                nc.gpsimd.tensor_tensor(out=tmp, in0=xtt, in1=rstd_bc, op=OP.mult)
                nc.vector.tensor_tensor(out=xnT_sl(ko), in0=tmp, in1=mun_bc, op=OP.add)

            # ---------------- Phase C: qkT = Wqk' @ xn^T ----------------
            wqk_sb = p_w1.tile([P, NT, 2 * HL * HD], F32R, tag="w1")
            nc.sync.dma_start(
                out=wqk_sb, in_=wqkT_d[:].rearrange("(ko p) m -> p ko m", p=P).bitcast(F32R)
            )
            bqk_sb = p_sm.tile([P, NT], F32, tag="bqk")
            nc.sync.dma_start(out=bqk_sb, in_=bqk_d[:].rearrange("(mt p) -> p mt", p=P))
            qkT_sb = p_qk.tile([P, NT, T], F32R, tag="qk")

            for mt in range(NT):
                psq = p_ps.tile([P, T], F32, tag="ps")
                for qh in range(2):
                    for ko in range(NT):
                        nc.tensor.matmul(
                            psq[:, qh * 512 : (qh + 1) * 512],
                            lhsT=wqk_sb[:, ko, mt * P : (mt + 1) * P],
                            rhs=xnT_sl(ko)[:, qh * 512 : (qh + 1) * 512],
                            start=(ko == 0),
                            stop=(ko == NT - 1),
                        )
                nc.scalar.activation(
                    out=qkT_sb[:, mt, :], in_=psq,
                    func=ACT.Identity, bias=bqk_sb[:, mt : mt + 1], scale=1.0,
                )

            # ---------------- Phase D: v (head-interleaved, ones col) ---
            wv_sb = p_w2.tile([P, NT, HL * HD], F32R, tag="w2")
            nc.sync.dma_start(
                out=wv_sb, in_=wvT_d[:].rearrange("(ko p) m -> p ko m", p=P).bitcast(F32R)
            )
            bv_bc = p_st.tile([P, HL * HD], F32, tag="bvbc")
            nc.sync.dma_start(
                out=bv_bc,
                in_=bass.AP(tensor=bv_d, offset=0, ap=[[0, P], [1, HL * HD]]),
            )
            # v_sb[p, c, 65h + j]: j<64 -> v head h dim j; j=64 -> 1.0
            v_sb = p_v.tile([P, NT, HL * 65], F32R, tag="vv")
            nc.vector.memset(
                v_sb.rearrange("p c (h j) -> p c h j", j=65)[:, :, :, 64].bitcast(F32),
                1.0,
            )
            for t in range(NT):
                psv = p_ps.tile([P, T], F32, tag="ps")
                for ko in range(NT):
                    nc.tensor.matmul(
                        psv[:, : HL * HD],
                        lhsT=xnT_sl(ko)[:, t * P : (t + 1) * P],
                        rhs=wv_sb[:, ko, :],
                        start=(ko == 0),
                        stop=(ko == NT - 1),
                    )
                nc.vector.tensor_tensor(
                    out=v_sb.rearrange("p c (h j) -> p c h j", j=65)[:, t, :, 0:64],
                    in0=psv[:, : HL * HD].rearrange("p (h j) -> p h j", j=HD),
                    in1=bv_bc.rearrange("p (h j) -> p h j", j=HD),
                    op=OP.add,
                )

            # ---------------- Phase E: attention per head ---------------
            seqb_sb = p_sm.tile([P, NT], F32, tag="seqb")
            nc.sync.dma_start(out=seqb_sb, in_=seqb_d[:])
            mix_sb = p_w2.tile([P, nmx, 512], F32, tag="mix")
            nc.sync.dma_start(
                out=mix_sb, in_=mix_d[:].rearrange("n p q -> p n q")
            )

            oT_on = p_big.tile([P, 4, T], F32R, tag="big")
            oT_off = p_big.tile([P, 4, T], F32R, tag="big")

            for h in range(HL):
                par = h % 2
                base = 64 * par
                qT_h = qkT_sb[base : base + 64, h // 2, :]
                kT_h = qkT_sb[base : base + 64, 4 + h // 2, :]
                vlo = 65 * h
                pon_t = p_po.tile([P, T], F32, tag="po")
                poff_t = p_po.tile([P, T], F32, tag="po")
                pon = pon_t[0:65]
                poff = poff_t[0:65]

                for ci, c in enumerate(used_chunks):
                    pss = p_ps.tile([P, T], F32, tag="ps")
                    for qh in range(2):
                        nc.tensor.matmul(
                            pss[:, qh * 512 : (qh + 1) * 512],
                            lhsT=kT_h[:, c * P : (c + 1) * P],
                            rhs=qT_h[:, qh * 512 : (qh + 1) * 512],
                            start=True,
                            stop=True,
                        )
                    pofc = p_p.tile([P, T], F32R, tag="pp")
                    nc.scalar.activation(
                        out=pofc, in_=pss, func=ACT.Exp,
                        bias=seqb_sb[:, c : c + 1], scale=1.0,
                    )
                    lhsT = v_sb[:, c, vlo : vlo + 65]
                    for qh in range(2):
                        nc.tensor.matmul(
                            poff[:, qh * 512 : (qh + 1) * 512],
                            lhsT=lhsT,
                            rhs=pofc[:, qh * 512 : (qh + 1) * 512],
                            start=(c == first_off),
                            stop=(c == last_off),
                        )
                    for qh in range(2):
                        k = cls[c][qh]
                        if k == 0 or first_on[qh] is None:
                            continue
                        qsl = slice(qh * 512, (qh + 1) * 512)
                        if k == 1:
                            rhs = pofc[:, qsl]
                        else:
                            nc.vector.tensor_tensor(
                                out=pss[:, qsl],
                                in0=pss[:, qsl],
                                in1=mix_sb[:, k - 2, :],
                                op=OP.add,
                            )
                            pblk = p_blk.tile([P, 512], F32R, tag="blk")
                            nc.scalar.activation(
                                out=pblk, in_=pss[:, qsl], func=ACT.Exp,
                                bias=seqb_sb[:, c : c + 1], scale=1.0,
                            )
                            rhs = pblk
                        nc.tensor.matmul(
                            pon[:, qsl],
                            lhsT=lhsT,
                            rhs=rhs,
                            start=(c == first_on[qh]),
                            stop=(c == last_on[qh]),
                        )

                # divide by the ones-column sums; DVE handles the partition
                # shift to this head's lanes (dlo) directly.
                dlo = base
                for pt, dst in ((pon_t, oT_on), (poff_t, oT_off)):
                    # single ACT copy frees the PSUM slot; the divide chain
                    # then runs from SBUF, overlapped with the next head's PE
                    otmp = p_bc.tile([P, T], F32, tag="bc")
                    nc.scalar.activation(out=otmp[0:65, :], in_=pt[0:65], func=ACT.Copy)
                    drow_dram = p_dram.tile([T], F32, tag="drow")
                    nc.sync.dma_start(out=drow_dram[None, :], in_=otmp[64:65, :])
                    rct = p_rc.tile([P, T], F32, tag="rc")
                    nc.sync.dma_start(
                        out=rct[64:128, :],
                        in_=bass.AP(
                            tensor=drow_dram.tensor,
                            offset=drow_dram.offset,
                            ap=[[0, 64], [1, T]],
                        ),
                    )
                    nc.vector.reciprocal(out=rct[0:64, :], in_=rct[64:128, :])
                    nc.vector.tensor_tensor(
                        out=dst[dlo : dlo + 64, h // 2, :],
                        in0=otmp[0:64],
                        in1=rct[0:64, :],
                        op=OP.mult,
                    )
                # zero any online q-halves no chunk wrote (fully masked)
                for qh in range(2):
                    if first_on[qh] is None:
                        nc.vector.memset(
                            oT_on[dlo : dlo + 64, h // 2, qh * 512 : (qh + 1) * 512]
                            .bitcast(F32),
                            0.0,
                        )

            # ---------------- Phase F: output projection ----------------
            wo_sb = p_w1.tile([P, 4, D], F32R, tag="w1")
            nc.sync.dma_start(
                out=wo_sb, in_=woT_d[:].rearrange("(j p) m -> p j m", p=P).bitcast(F32R)
            )
            for src, dst_d in ((oT_on, oon_d), (oT_off, ooff_d)):
                for t in range(NT):
                    pso = p_ps.tile([P, T], F32, tag="ps")
                    for dh in range(2):
                        for j in range(4):
                            nc.tensor.matmul(
                                pso[:, dh * 512 : (dh + 1) * 512],
                                lhsT=src[:, j, t * P : (t + 1) * P],
                                rhs=wo_sb[:, j, dh * 512 : (dh + 1) * 512],
                                start=(j == 0),
                                stop=(j == 3),
                            )
                    ot = p_io.tile([P, D], F32, tag="io")
                    nc.scalar.activation(out=ot, in_=pso, func=ACT.Copy)
                    nc.sync.dma_start(out=dst_d[t * P : (t + 1) * P, :], in_=ot)

    _split_multi_waits(nc)
    return nc


def _get_program(key, used_chunks, cls, n_mixed):
    if key not in _prog_cache:
        _install_patches()
        _prog_cache[key] = _build_program(used_chunks, cls, n_mixed)
    return _prog_cache[key]


def kernel(
    input_tensor,
    ln_gamma,
    ln_beta,
    in_proj_w,
    in_proj_b,
    out_w,
    out_b,
    sequence_mask,
    attn_mask,
):
    x = np.asarray(input_tensor, np.float32)
    gamma = np.asarray(ln_gamma, np.float32)
    beta = np.asarray(ln_beta, np.float32)
    W = np.asarray(in_proj_w, np.float32)
    bias = np.asarray(in_proj_b, np.float32)
    Wo = np.asarray(out_w, np.float32)
    bo = np.asarray(out_b, np.float32)
    seqm = np.asarray(sequence_mask, bool)
    attn = np.asarray(attn_mask, bool)

    # ---- mask-derived program structure (identical across cores) ----
    used_chunks = [
        c for c in range(NT) if seqm[:, c * P : (c + 1) * P].any()
    ] or [0]
    attnT = attn.T
    cls, mixed = _classify_blocks(attnT)
    key = (tuple(used_chunks), tuple(tuple(r) for r in cls))
    nc = _get_program(key, used_chunks, cls, len(mixed))

    if mixed:
        mix_arr = np.stack(mixed, axis=0)
    else:
        mix_arr = np.zeros((1, P, 512), np.float32)

    # ---- host-side weight folding (gamma/beta/scale into W, b) ----
    scale_q = 1.0 / np.sqrt(HD)
    Wg = W * gamma[None, :]          # fold gamma
    bfold = bias + W @ beta          # fold beta
    in_maps = []
    for c in range(8):
        b = c // 2
        g = c % 2
        qs, ks, vs = 512 * g, D + 512 * g, 2 * D + 512 * g
        wq = Wg[qs : qs + 512] * scale_q
        wk = Wg[ks : ks + 512]
        wv = Wg[vs : vs + 512]
        bq = bfold[qs : qs + 512] * scale_q
        bk = bfold[ks : ks + 512]
        bv = bfold[vs : vs + 512]
        wqkT = np.ascontiguousarray(np.concatenate([wq, wk], axis=0).T)
        seqb = np.where(seqm[b], 0.0, NEG).astype(np.float32) - C_SHIFT
        in_maps.append(
            {
                "x": np.ascontiguousarray(x[b]),
                "xT": np.ascontiguousarray(x[b].T),
                "wqkT": wqkT,
                "wvT": np.ascontiguousarray(wv.T),
                "woT": np.ascontiguousarray(Wo[:, 512 * g : 512 * g + 512].T),
                "bqk": np.ascontiguousarray(np.concatenate([bq, bk])),
                "bv": np.ascontiguousarray(bv),
                "seqb": np.ascontiguousarray(seqb.reshape(NT, P).T),
                "mix": mix_arr,
            }
        )

    global _last_in_maps
    _last_in_maps = in_maps
    res = run_bass_kernel_spmd(nc, in_maps, list(range(8)))

    out = np.empty((2, B, T, D), np.float32)
    for b in range(B):
        r0, r1 = res.results[2 * b], res.results[2 * b + 1]
        out[0, b] = r0["out_on"] + r1["out_on"] + bo[None, :]
        out[1, b] = r0["out_off"] + r1["out_off"] + bo[None, :]
    return out
